# revision 29
# baseline (speedup 1.0000x reference)
"""AlphaRotatedGIoULoss on 8 TRN2 NeuronCores.

Data-parallel: 500000 box pairs sharded 62500/core, laid out as
(125 partitions x 500 boxes). Per-box rotated-GIoU via a branchless
line-integral intersection (slab clipping in each box's axis-aligned
frame + a frame-change correction term), so no sorting/gather is needed.
Output: per-core partial loss sums (125,1); host sums and divides.
"""
import sys
import numpy as np

for _p in ("/opt/trn_rl_repo", "/root/.axon_site/_ro/trn_rl_repo"):
    if _p not in sys.path:
        sys.path.insert(0, _p)

N_CORES = 8
N_TOTAL = 500000
N_CORE = N_TOTAL // N_CORES   # 62500
P = 125                       # partitions used
FB = 500                      # boxes per partition row (125*500 = 62500)
NT = 2                        # column tiles
FT = FB // NT                 # boxes per column tile
PI_2 = 1.5707963267948966

_CACHE = {}


def _build():
    import concourse.bass as bass  # noqa: F401
    import concourse.bacc as bacc
    import concourse.tile as tile
    from concourse import mybir

    f32 = mybir.dt.float32
    AF = mybir.ActivationFunctionType
    OP = mybir.AluOpType
    AX_ = mybir.AxisListType

    import os
    debug = bool(os.environ.get("K_DEBUG"))
    nc = bacc.Bacc(None, target_bir_lowering=False)
    pred_d = nc.declare_dram_parameter("pred", [N_CORE, 5], f32, isOutput=False)
    tgt_d = nc.declare_dram_parameter("target", [N_CORE, 5], f32, isOutput=False)
    out_d = nc.declare_dram_parameter("out", [P, 1], f32, isOutput=True)
    dbg_d = None
    if debug:
        dbg_d = nc.declare_dram_parameter("dbg", [4, P, FB], f32, isOutput=True)

    V = nc.vector
    S = nc.scalar
    G = nc.gpsimd

    def vtt(out, a, b, op):
        V.tensor_tensor(out, a, b, op)

    def gtt(out, a, b, op):
        # GpSimd elementwise proved both slower (Q7 per-instruction overhead
        # at these tile widths) and unreliable here -> everything on VectorE
        V.tensor_tensor(out, a, b, op)

    from contextlib import ExitStack

    with tile.TileContext(nc) as tc:
        with (
            tc.tile_pool(name="pre", bufs=1) as pre,
            tc.tile_pool(name="small", bufs=1) as sm,
            ExitStack() as stack,
        ):
            io = stack.enter_context(tc.tile_pool(name="io", bufs=1))
            comb = io.tile([P, 2 * FB * 5], f32, tag="comb")
            pio2 = sm.tile([P, 1], f32, tag="pio2")
            V.memset(pio2[:], PI_2)
            cv = comb[:].rearrange("p (h f c) -> p h f c", h=2, c=5)
            # halves of comb: h=0 pred, h=1 target
            # (an 8-way partition-row DMA split was tried and measured SLOWER:
            # 32-row chunks engage only a quarter of the SBUF ports each)
            nc.sync.dma_start(out=cv[:, 0], in_=pred_d.rearrange("(p f) c -> p f c", p=P))
            nc.sync.dma_start(out=cv[:, 1], in_=tgt_d.rearrange("(p f) c -> p f c", p=P))

            def feat(h, i):       # (P, FB) plain feature plane view
                return cv[:, h, :, i]

            def featS(i):         # (P, 2, FB) stacked [pred|target]
                return cv[:, :, :, i]

            # stacked planes: physical (P, 2*FB); half 0 = frame-B terms
            # (A's geometry clipped by target box B), half 1 = frame-A terms.
            class SP:
                def __init__(self, name, w=FB):
                    self.w = w
                    self.t = pre.tile([P, 2 * w], f32, tag=name)

                def full(self):
                    return self.t[:]

                def h(self, i):
                    return self.t[:, i * self.w:(i + 1) * self.w]

                def sl(self, c0, n):   # (P,2,n) column slice of both halves
                    return self.t[:].rearrange("p (h f) -> p h f", h=2)[:, :, c0:c0 + n]

                def hsl(self, i, c0, n):
                    return self.t[:, i * self.w + c0: i * self.w + c0 + n]

            ddxS, ddyS, dlt = SP("ddxS"), SP("ddyS"), SP("dlt")
            sdS, cdS, cS, sS = SP("sdS"), SP("cdS"), SP("cS"), SP("sS")
            t1p, t2p = SP("t1p"), SP("t2p")
            dX, dY = SP("dX"), SP("dY")
            whS, hhS = SP("whS"), SP("hhS")
            wc, ws, hs, hc = SP("wc"), SP("ws"), SP("hs"), SP("hc")
            g0x, g0y, n1, n2 = SP("g0x"), SP("g0y"), SP("n1"), SP("n2")
            Wc, Hc = SP("Wc"), SP("Hc")
            rwc, rws, rhs, rhc = SP("rwc"), SP("rws"), SP("rhs"), SP("rhc")

            # ---- pre-pass (full width) ----
            gtt(ddxS.h(0), feat(0, 0), feat(1, 0), OP.subtract)   # x1-x2
            gtt(ddxS.h(1), feat(1, 0), feat(0, 0), OP.subtract)   # x2-x1
            gtt(ddyS.h(0), feat(0, 1), feat(1, 1), OP.subtract)
            gtt(ddyS.h(1), feat(1, 1), feat(0, 1), OP.subtract)
            vtt(dlt.h(0), feat(0, 4), feat(1, 4), OP.subtract)    # a1-a2
            vtt(dlt.h(1), feat(1, 4), feat(0, 4), OP.subtract)
            # all Sin activations batched (one table set)
            S.activation(sdS.full(), dlt.full(), AF.Sin)                 # [sd|-sd]
            # cos(dlt) = sin(dlt + pi/2); dlt+pi/2 can exceed pi where the
            # Sin table degrades -> wrap into [-pi, pi] first
            V.add_range_wrap(cdS.full(), dlt.full(), PI_2, 3.141592653589793,
                             6.283185307179586)
            S.activation(cdS.full(), cdS.full(), AF.Sin)                 # [cd|cd]
            S.activation(cS.h(0), feat(1, 4), AF.Sin, bias=pio2[:])      # c2
            S.activation(cS.h(1), feat(0, 4), AF.Sin, bias=pio2[:])      # c1
            S.activation(sS.h(0), feat(1, 4), AF.Sin)                    # s2
            S.activation(sS.h(1), feat(0, 4), AF.Sin)                    # s1
            # delta = R^T * (center difference), stacked (GpSimd chain)
            gtt(t1p.full(), cS.full(), ddxS.full(), OP.mult)
            gtt(t2p.full(), sS.full(), ddyS.full(), OP.mult)
            gtt(dX.full(), t1p.full(), t2p.full(), OP.add)
            gtt(t1p.full(), cS.full(), ddyS.full(), OP.mult)
            gtt(t2p.full(), sS.full(), ddxS.full(), OP.mult)
            gtt(dY.full(), t1p.full(), t2p.full(), OP.subtract)
            # half dims of the moving box: [w1|w2]/2, [h1|h2]/2
            S.activation(whS.full(), featS(2), AF.Copy, scale=0.5)
            S.activation(hhS.full(), featS(3), AF.Copy, scale=0.5)
            vtt(wc.full(), whS.full(), cdS.full(), OP.mult)
            vtt(ws.full(), whS.full(), sdS.full(), OP.mult)
            vtt(hs.full(), hhS.full(), sdS.full(), OP.mult)
            vtt(hc.full(), hhS.full(), cdS.full(), OP.mult)
            gtt(g0x.full(), wc.full(), hs.full(), OP.subtract)
            gtt(g0y.full(), ws.full(), hc.full(), OP.add)
            gtt(n1.full(), wc.full(), hs.full(), OP.add)          # -g1x
            gtt(n2.full(), hc.full(), ws.full(), OP.subtract)     # g1y
            # clip half-extents of the fixed box: [w2|w1]/2, [h2|h1]/2 (+neg)
            S.activation(Wc.h(0), feat(1, 2), AF.Copy, scale=0.5)
            S.activation(Wc.h(1), feat(0, 2), AF.Copy, scale=0.5)
            S.activation(Hc.h(0), feat(1, 3), AF.Copy, scale=0.5)
            S.activation(Hc.h(1), feat(0, 3), AF.Copy, scale=0.5)
            nWc, nHc = SP("nWc"), SP("nHc")
            S.activation(nWc.h(0), feat(1, 2), AF.Copy, scale=-0.5)
            S.activation(nWc.h(1), feat(0, 2), AF.Copy, scale=-0.5)
            S.activation(nHc.h(0), feat(1, 3), AF.Copy, scale=-0.5)
            S.activation(nHc.h(1), feat(0, 3), AF.Copy, scale=-0.5)
            # reciprocals of edge direction components; the +1e-20 only
            # rescues an exact-zero denominator (parallel edges) from NaN
            for rp, src in ((rwc, wc), (rws, ws), (rhs, hs), (rhc, hc)):
                V.tensor_scalar(rp.full(), src.full(), 2.0, 1e-20, OP.mult, OP.add)
                V.reciprocal_approx_fast(out=rp.full(), in_=rp.full())
            # union0 = w1*h1 + w2*h2  (plain width FB)
            m1 = io.tile([P, FB], f32, tag="m1")
            m2 = io.tile([P, FB], f32, tag="m2")
            union0 = sm.tile([P, FB], f32, tag="union0")
            gtt(m1[:], feat(0, 2), feat(0, 3), OP.mult)
            gtt(m2[:], feat(1, 2), feat(1, 3), OP.mult)
            gtt(union0[:], m1[:], m2[:], OP.add)

            # input tile + prepass scratch no longer needed: free the io pool
            # so the heavy per-column-tile pool can use its SBUF space
            stack.close()
            hv = stack.enter_context(tc.tile_pool(name="heavy", bufs=1))

            lsums = []
            SW = 2 * FT  # stacked width per edge slice

            for t in range(NT):
                c0 = t * FT

                def E(tile4):     # (P, 4, 2, FT) edge/half view of 4*SW tile
                    return tile4[:].rearrange("p (e h f) -> p e h f", e=4, h=2)

                AXt = hv.tile([P, 4 * SW], f32, tag="AXt")
                AYt = hv.tile([P, 4 * SW], f32, tag="AYt")
                INX = hv.tile([P, 4 * SW], f32, tag="INX")
                INY = hv.tile([P, 4 * SW], f32, tag="INY")
                Ut = hv.tile([P, 4 * SW], f32, tag="Ut")
                Vt = hv.tile([P, 4 * SW], f32, tag="Vt")
                NPt = hv.tile([P, 4 * SW], f32, tag="NPt")
                TLX = hv.tile([P, 4 * SW], f32, tag="TLX")

                dXc, dYc = dX.sl(c0, FT), dY.sl(c0, FT)
                g0xc, g0yc = g0x.sl(c0, FT), g0y.sl(c0, FT)
                n1c, n2c = n1.sl(c0, FT), n2.sl(c0, FT)
                Wcc, Hcc = Wc.sl(c0, FT), Hc.sl(c0, FT)

                # corners of the moving box in the fixed box's frame (GpSimd)
                gtt(E(AXt)[:, 0], dXc, g0xc, OP.add)
                gtt(E(AXt)[:, 1], dXc, n1c, OP.subtract)
                gtt(E(AXt)[:, 2], dXc, g0xc, OP.subtract)
                gtt(E(AXt)[:, 3], dXc, n1c, OP.add)
                gtt(E(AYt)[:, 0], dYc, g0yc, OP.add)
                gtt(E(AYt)[:, 1], dYc, n2c, OP.add)
                gtt(E(AYt)[:, 2], dYc, g0yc, OP.subtract)
                gtt(E(AYt)[:, 3], dYc, n2c, OP.subtract)

                # ---- enclosing rect (bbox in each frame, min of the two) ----
                exm = sm.tile([P, SW], f32, tag="exm")
                exn = sm.tile([P, SW], f32, tag="exn")
                exs = sm.tile([P, SW], f32, tag="exs")
                eys = sm.tile([P, SW], f32, tag="eys")
                ex3 = exm[:].rearrange("p (h f) -> p h f", h=2)
                en3 = exn[:].rearrange("p (h f) -> p h f", h=2)
                es3 = exs[:].rearrange("p (h f) -> p h f", h=2)
                ey3 = eys[:].rearrange("p (h f) -> p h f", h=2)
                # (min/max TT are not legal on GpSimd; comparisons stay on V,
                # adds/mults go to GpSimd)
                for crn, ext, clamp, dst3 in ((AXt, exs, Wcc, es3), (AYt, eys, Hcc, ey3)):
                    c4 = E(crn)
                    vtt(ex3, c4[:, 0], c4[:, 1], OP.max)
                    vtt(en3, c4[:, 2], c4[:, 3], OP.max)
                    vtt(ex3, ex3, en3, OP.max)                    # mx
                    vtt(dst3, ex3, clamp, OP.max)                 # max(mx, W)
                    vtt(ex3, c4[:, 0], c4[:, 1], OP.min)
                    vtt(en3, c4[:, 2], c4[:, 3], OP.min)
                    vtt(ex3, ex3, en3, OP.min)                    # mn
                    S.activation(ex3, ex3, AF.Copy, scale=-1.0)   # -mn
                    vtt(ex3, ex3, clamp, OP.max)                  # max(-mn, W)
                    gtt(dst3, dst3, ex3, OP.add)                  # extent
                gtt(exs[:], exs[:], eys[:], OP.mult)              # areaC stacked
                area_c = sm.tile([P, FT], f32, tag="area_c")
                vtt(area_c[:], es3[:, 0], es3[:, 1], OP.min)

                # ---- signed reciprocal planes (edges 0,1 only: edges 2,3
                # come from the point symmetry a2 = 2*delta - a0, d2 = -d0,
                # whose slab roots are m + roots(edge0), m = 2*delta*inv) ----
                rwcc, rwsc = rwc.sl(c0, FT), rws.sl(c0, FT)
                rhsc, rhcc = rhs.sl(c0, FT), rhc.sl(c0, FT)
                for dst, srcs in (
                    (INX, ((rwcc, -1.0), (rhsc, 1.0))),
                    (INY, ((rwsc, -1.0), (rhcc, -1.0))),
                ):
                    d4 = E(dst)
                    for e, (src, sc) in enumerate(srcs):
                        S.activation(d4[:, e], src, AF.Copy, scale=sc)
                t2d = sm.tile([P, SW], f32, tag="t2d")
                t2d3 = t2d[:].rearrange("p (h f) -> p h f", h=2)
                HW2 = 2 * SW

                def H01(t4):
                    return t4[:, 0:HW2]

                def H23(t4):
                    return t4[:, HW2:2 * HW2]

                # ---- slab clip, x axis, edges 0,1 ----
                nWcc, nHcc = nWc.sl(c0, FT), nHc.sl(c0, FT)
                for e in range(2):
                    vtt(E(Ut)[:, e], nWcc, E(AXt)[:, e], OP.subtract)  # -W - ax
                    vtt(E(Vt)[:, e], Wcc, E(AXt)[:, e], OP.subtract)   # W - ax
                vtt(H01(Ut), H01(Ut), H01(INX), OP.mult)               # ta01
                vtt(H01(Vt), H01(Vt), H01(INX), OP.mult)               # tb01
                vtt(H01(TLX), H01(Ut), H01(Vt), OP.min)                # tlo01
                vtt(H01(Ut), H01(Ut), H01(Vt), OP.max)                 # thi01
                S.activation(t2d3, dXc, AF.Copy, scale=2.0)            # 2*dx
                vtt(E(Vt)[:, 0], t2d3, E(INX)[:, 0], OP.mult)          # m0
                vtt(E(Vt)[:, 1], t2d3, E(INX)[:, 1], OP.mult)          # m1
                vtt(H23(TLX), H01(Vt), H01(TLX), OP.add)               # tlo23
                vtt(H23(Ut), H01(Vt), H01(Ut), OP.add)                 # thi23
                # ---- slab clip, y axis, edges 0,1 ----
                for e in range(2):
                    vtt(E(Vt)[:, e], nHcc, E(AYt)[:, e], OP.subtract)  # -H - ay
                    vtt(E(NPt)[:, e], Hcc, E(AYt)[:, e], OP.subtract)  # H - ay
                vtt(H01(Vt), H01(Vt), H01(INY), OP.mult)               # ta01_y
                vtt(H01(NPt), H01(NPt), H01(INY), OP.mult)             # tb01_y
                vtt(H01(INX), H01(Vt), H01(NPt), OP.min)               # tlo01_y
                vtt(H01(Vt), H01(Vt), H01(NPt), OP.max)                # thi01_y
                S.activation(t2d3, dYc, AF.Copy, scale=2.0)            # 2*dy
                vtt(E(NPt)[:, 0], t2d3, E(INY)[:, 0], OP.mult)         # m0_y
                vtt(E(NPt)[:, 1], t2d3, E(INY)[:, 1], OP.mult)         # m1_y
                vtt(H23(INX), H01(NPt), H01(INX), OP.add)              # tlo23_y
                vtt(H23(Vt), H01(NPt), H01(Vt), OP.add)                # thi23_y
                # ---- interval intersect, dt ----
                # t0 = max(tlo_x, tlo_y, 0); t1 = min(thi_x, thi_y, 1)
                V.scalar_tensor_tensor(TLX[:], TLX[:], 0.0, INX[:], OP.max, OP.max)
                V.scalar_tensor_tensor(Ut[:], Ut[:], 1.0, Vt[:], OP.min, OP.min)
                vtt(TLX[:], Ut[:], TLX[:], OP.subtract)                # t1-t0
                S.activation(TLX[:], TLX[:], AF.Relu)                  # dt
                # ---- direction planes, cross(a,d), pieces ----
                wcc, wsc = wc.sl(c0, FT), ws.sl(c0, FT)
                hsc, hcc = hs.sl(c0, FT), hc.sl(c0, FT)
                for dst, srcs in (
                    (INX, ((wcc, -2.0), (hsc, 2.0), (wcc, 2.0), (hsc, -2.0))),   # dx
                    (INY, ((wsc, -2.0), (hcc, -2.0), (wsc, 2.0), (hcc, 2.0))),   # dy
                ):
                    d4 = E(dst)
                    for e, (src, sc) in enumerate(srcs):
                        S.activation(d4[:, e], src, AF.Copy, scale=sc)
                vtt(Vt[:], AXt[:], INY[:], OP.mult)                    # ax*dy
                vtt(Ut[:], AYt[:], INX[:], OP.mult)                    # ay*dx
                vtt(Vt[:], Vt[:], Ut[:], OP.subtract)                  # cad
                vtt(Ut[:], TLX[:], Vt[:], OP.mult)                     # pieces

                # ---- piece sum (stacked), SA (frame-B half) ----
                psS = sm.tile([P, SW], f32, tag="psS")
                ps3 = psS[:].rearrange("p (h f) -> p h f", h=2)
                u4 = E(Ut)
                gtt(ps3, u4[:, 0], u4[:, 1], OP.add)
                gtt(es3, u4[:, 2], u4[:, 3], OP.add)                   # reuse exs
                gtt(ps3, ps3, es3, OP.add)
                dt4 = E(TLX)
                sax = sm.tile([P, FT], f32, tag="sax")
                say = sm.tile([P, FT], f32, tag="say")
                sau = sm.tile([P, FT], f32, tag="sau")
                sav = sm.tile([P, FT], f32, tag="sav")
                st1 = sm.tile([P, FT], f32, tag="st1")
                gtt(sau[:], dt4[:, 2, 0], dt4[:, 0, 0], OP.subtract)
                gtt(sav[:], dt4[:, 3, 0], dt4[:, 1, 0], OP.subtract)
                dx4, dy4 = E(INX), E(INY)
                gtt(sax[:], dx4[:, 2, 0], sau[:], OP.mult)
                gtt(st1[:], dx4[:, 3, 0], sav[:], OP.mult)
                gtt(sax[:], sax[:], st1[:], OP.add)
                gtt(say[:], dy4[:, 2, 0], sau[:], OP.mult)
                gtt(st1[:], dy4[:, 3, 0], sav[:], OP.mult)
                gtt(say[:], say[:], st1[:], OP.add)
                # corr = ddy*(c2*sax - s2*say)... rotated by R2:
                # RSx = c2*sax - s2*say ; RSy = s2*sax + c2*say
                c2v = cS.hsl(0, c0, FT)
                s2v = sS.hsl(0, c0, FT)
                rsx = sm.tile([P, FT], f32, tag="rsx")
                rsy = sm.tile([P, FT], f32, tag="rsy")
                gtt(rsx[:], c2v, sax[:], OP.mult)
                gtt(st1[:], s2v, say[:], OP.mult)
                gtt(rsx[:], rsx[:], st1[:], OP.subtract)
                gtt(rsy[:], s2v, sax[:], OP.mult)
                gtt(st1[:], c2v, say[:], OP.mult)
                gtt(rsy[:], rsy[:], st1[:], OP.add)
                inter = sm.tile([P, FT], f32, tag="inter")
                gtt(inter[:], ddyS.hsl(0, c0, FT), rsx[:], OP.mult)
                gtt(st1[:], ddxS.hsl(0, c0, FT), rsy[:], OP.mult)
                gtt(inter[:], inter[:], st1[:], OP.subtract)           # corr
                gtt(inter[:], inter[:], ps3[:, 0], OP.add)
                gtt(inter[:], inter[:], ps3[:, 1], OP.add)
                S.activation(inter[:], inter[:], AF.Relu, scale=0.5)   # inter area

                # ---- final loss ----
                union = sm.tile([P, FT], f32, tag="union")
                gtt(union[:], union0[:, c0:c0 + FT], inter[:], OP.subtract)
                iou = sm.tile([P, FT], f32, tag="iou")
                V.reciprocal_approx_fast(out=st1[:], in_=union[:])
                gtt(iou[:], inter[:], st1[:], OP.mult)
                V.tensor_scalar(iou[:], iou[:], 1e-6, None, OP.max)
                V.reciprocal_approx_fast(out=st1[:], in_=area_c[:])
                gtt(st1[:], union[:], st1[:], OP.mult)
                rr = sm.tile([P, FT], f32, tag="rr")
                V.tensor_scalar(rr[:], st1[:], -1.0, 1.0, OP.mult, OP.add)  # 1-u/ac
                gtt(st1[:], iou[:], iou[:], OP.mult)
                gtt(st1[:], st1[:], iou[:], OP.mult)                   # iou^3
                gtt(iou[:], rr[:], rr[:], OP.mult)
                gtt(iou[:], iou[:], rr[:], OP.mult)                    # r^3
                gtt(st1[:], st1[:], iou[:], OP.subtract)               # giou
                V.tensor_scalar(st1[:], st1[:], -1.0, 1.0, OP.mult, OP.add)
                ls = sm.tile([P, 1], f32, tag=f"ls{t}")
                V.tensor_reduce(ls[:], st1[:], AX_.X, OP.add)
                if debug:
                    nc.sync.dma_start(out=dbg_d[0, :, c0:c0 + FT], in_=st1[:])
                    nc.sync.dma_start(out=dbg_d[1, :, c0:c0 + FT], in_=inter[:])
                    nc.sync.dma_start(out=dbg_d[2, :, c0:c0 + FT], in_=union[:])
                    nc.sync.dma_start(out=dbg_d[3, :, c0:c0 + FT], in_=area_c[:])
                lsums.append(ls)

            acc = sm.tile([P, 1], f32, tag="acc")
            gtt(acc[:], lsums[0][:], lsums[1][:], OP.add)
            nc.sync.dma_start(out=out_d[:], in_=acc[:])

    nc.finalize()
    return nc


def _get_nc():
    if "nc" not in _CACHE:
        _CACHE["nc"] = _build()
    return _CACHE["nc"]


def kernel(pred, target):
    from concourse.bass_utils import run_bass_kernel_spmd

    pred = np.ascontiguousarray(np.asarray(pred, dtype=np.float32))
    target = np.ascontiguousarray(np.asarray(target, dtype=np.float32))
    nc = _get_nc()
    in_maps = []
    for i in range(N_CORES):
        sl = slice(i * N_CORE, (i + 1) * N_CORE)
        in_maps.append({"pred": pred[sl], "target": target[sl]})
    res = run_bass_kernel_spmd(nc, in_maps, core_ids=list(range(N_CORES)))
    total = np.float64(0.0)
    for i in range(N_CORES):
        total += np.asarray(res.results[i]["out"], dtype=np.float64).sum()
    return np.float32(total / N_TOTAL)


# revision 30
# speedup vs baseline: 1.1881x; 1.1881x over previous
"""AlphaRotatedGIoULoss on 8 TRN2 NeuronCores.

Data-parallel: 500000 box pairs sharded 62500/core, laid out as
(125 partitions x 500 boxes). Per-box rotated-GIoU via a branchless
line-integral intersection (slab clipping in each box's axis-aligned
frame + a frame-change correction term), so no sorting/gather is needed.
Output: per-core partial loss sums (125,1); host sums and divides.
"""
import sys
import numpy as np

for _p in ("/opt/trn_rl_repo", "/root/.axon_site/_ro/trn_rl_repo"):
    if _p not in sys.path:
        sys.path.insert(0, _p)

N_CORES = 8
N_TOTAL = 500000
N_CORE = N_TOTAL // N_CORES   # 62500
P = 125                       # partitions used
FB = 500                      # boxes per partition row (125*500 = 62500)
NT = 2                        # column tiles
FT = FB // NT                 # boxes per column tile
PI_2 = 1.5707963267948966

_CACHE = {}


def _build():
    import concourse.bass as bass  # noqa: F401
    import concourse.bacc as bacc
    import concourse.tile as tile
    from concourse import mybir

    f32 = mybir.dt.float32
    AF = mybir.ActivationFunctionType
    OP = mybir.AluOpType
    AX_ = mybir.AxisListType

    import os
    debug = bool(os.environ.get("K_DEBUG"))
    nc = bacc.Bacc(None, target_bir_lowering=False)
    pred_d = nc.declare_dram_parameter("pred", [N_CORE, 5], f32, isOutput=False)
    tgt_d = nc.declare_dram_parameter("target", [N_CORE, 5], f32, isOutput=False)
    out_d = nc.declare_dram_parameter("out", [P, 1], f32, isOutput=True)
    dbg_d = None
    if debug:
        dbg_d = nc.declare_dram_parameter("dbg", [4, P, FB], f32, isOutput=True)

    V = nc.vector
    S = nc.scalar
    G = nc.gpsimd

    def vtt(out, a, b, op):
        V.tensor_tensor(out, a, b, op)

    def gtt(out, a, b, op):
        # GpSimd elementwise proved both slower (Q7 per-instruction overhead
        # at these tile widths) and unreliable here -> everything on VectorE
        V.tensor_tensor(out, a, b, op)

    from contextlib import ExitStack

    with tile.TileContext(nc) as tc:
        with (
            tc.tile_pool(name="pre", bufs=1) as pre,
            tc.tile_pool(name="small", bufs=1) as sm,
            ExitStack() as stack,
        ):
            io = stack.enter_context(tc.tile_pool(name="io", bufs=1))
            comb = io.tile([P, 2 * FB * 5], f32, tag="comb")
            pio2 = sm.tile([P, 1], f32, tag="pio2")
            V.memset(pio2[:], PI_2)
            cv = comb[:].rearrange("p (h f c) -> p h f c", h=2, c=5)
            # halves of comb: h=0 pred, h=1 target
            # (an 8-way partition-row DMA split was tried and measured SLOWER:
            # 32-row chunks engage only a quarter of the SBUF ports each)
            nc.sync.dma_start(out=cv[:, 0], in_=pred_d.rearrange("(p f) c -> p f c", p=P))
            nc.sync.dma_start(out=cv[:, 1], in_=tgt_d.rearrange("(p f) c -> p f c", p=P))

            def feat(h, i):       # (P, FB) plain feature plane view
                return cv[:, h, :, i]

            def featS(i):         # (P, 2, FB) stacked [pred|target]
                return cv[:, :, :, i]

            # stacked planes: physical (P, 2*FB); half 0 = frame-B terms
            # (A's geometry clipped by target box B), half 1 = frame-A terms.
            class SP:
                def __init__(self, name, w=FB):
                    self.w = w
                    self.t = pre.tile([P, 2 * w], f32, tag=name)

                def full(self):
                    return self.t[:]

                def h(self, i):
                    return self.t[:, i * self.w:(i + 1) * self.w]

                def sl(self, c0, n):   # (P,2,n) column slice of both halves
                    return self.t[:].rearrange("p (h f) -> p h f", h=2)[:, :, c0:c0 + n]

                def hsl(self, i, c0, n):
                    return self.t[:, i * self.w + c0: i * self.w + c0 + n]

            ddxS, ddyS, dlt = SP("ddxS"), SP("ddyS"), SP("dlt")
            sdS, cdS, cS, sS = SP("sdS"), SP("cdS"), SP("cS"), SP("sS")
            t1p, t2p = SP("t1p"), SP("t2p")
            dX, dY = SP("dX"), SP("dY")
            whS, hhS = SP("whS"), SP("hhS")
            wc, ws, hs, hc = SP("wc"), SP("ws"), SP("hs"), SP("hc")
            g0x, g0y, n1, n2 = SP("g0x"), SP("g0y"), SP("n1"), SP("n2")
            Wc, Hc = SP("Wc"), SP("Hc")
            rwc, rws, rhs, rhc = SP("rwc"), SP("rws"), SP("rhs"), SP("rhc")

            # ---- pre-pass (full width) ----
            gtt(ddxS.h(0), feat(0, 0), feat(1, 0), OP.subtract)   # x1-x2
            gtt(ddxS.h(1), feat(1, 0), feat(0, 0), OP.subtract)   # x2-x1
            gtt(ddyS.h(0), feat(0, 1), feat(1, 1), OP.subtract)
            gtt(ddyS.h(1), feat(1, 1), feat(0, 1), OP.subtract)
            vtt(dlt.h(0), feat(0, 4), feat(1, 4), OP.subtract)    # a1-a2
            vtt(dlt.h(1), feat(1, 4), feat(0, 4), OP.subtract)
            # all Sin activations batched (one table set)
            S.activation(sdS.full(), dlt.full(), AF.Sin)                 # [sd|-sd]
            # cos(dlt) = sin(dlt + pi/2); dlt+pi/2 can exceed pi where the
            # Sin table degrades -> wrap into [-pi, pi] first
            V.add_range_wrap(cdS.full(), dlt.full(), PI_2, 3.141592653589793,
                             6.283185307179586)
            S.activation(cdS.full(), cdS.full(), AF.Sin)                 # [cd|cd]
            S.activation(cS.h(0), feat(1, 4), AF.Sin, bias=pio2[:])      # c2
            S.activation(cS.h(1), feat(0, 4), AF.Sin, bias=pio2[:])      # c1
            S.activation(sS.h(0), feat(1, 4), AF.Sin)                    # s2
            S.activation(sS.h(1), feat(0, 4), AF.Sin)                    # s1
            # delta = R^T * (center difference), stacked (GpSimd chain)
            gtt(t1p.full(), cS.full(), ddxS.full(), OP.mult)
            gtt(t2p.full(), sS.full(), ddyS.full(), OP.mult)
            gtt(dX.full(), t1p.full(), t2p.full(), OP.add)
            gtt(t1p.full(), cS.full(), ddyS.full(), OP.mult)
            gtt(t2p.full(), sS.full(), ddxS.full(), OP.mult)
            gtt(dY.full(), t1p.full(), t2p.full(), OP.subtract)
            # half dims of the moving box: [w1|w2]/2, [h1|h2]/2
            S.activation(whS.full(), featS(2), AF.Copy, scale=0.5)
            S.activation(hhS.full(), featS(3), AF.Copy, scale=0.5)
            vtt(wc.full(), whS.full(), cdS.full(), OP.mult)
            vtt(ws.full(), whS.full(), sdS.full(), OP.mult)
            vtt(hs.full(), hhS.full(), sdS.full(), OP.mult)
            vtt(hc.full(), hhS.full(), cdS.full(), OP.mult)
            gtt(g0x.full(), wc.full(), hs.full(), OP.subtract)
            gtt(g0y.full(), ws.full(), hc.full(), OP.add)
            gtt(n1.full(), wc.full(), hs.full(), OP.add)          # -g1x
            gtt(n2.full(), hc.full(), ws.full(), OP.subtract)     # g1y
            # clip half-extents of the fixed box: [w2|w1]/2, [h2|h1]/2 (+neg)
            S.activation(Wc.h(0), feat(1, 2), AF.Copy, scale=0.5)
            S.activation(Wc.h(1), feat(0, 2), AF.Copy, scale=0.5)
            S.activation(Hc.h(0), feat(1, 3), AF.Copy, scale=0.5)
            S.activation(Hc.h(1), feat(0, 3), AF.Copy, scale=0.5)
            nWc, nHc = SP("nWc"), SP("nHc")
            S.activation(nWc.h(0), feat(1, 2), AF.Copy, scale=-0.5)
            S.activation(nWc.h(1), feat(0, 2), AF.Copy, scale=-0.5)
            S.activation(nHc.h(0), feat(1, 3), AF.Copy, scale=-0.5)
            S.activation(nHc.h(1), feat(0, 3), AF.Copy, scale=-0.5)
            # reciprocals of edge direction components; the +1e-20 only
            # rescues an exact-zero denominator (parallel edges) from NaN
            for rp, src in ((rwc, wc), (rws, ws), (rhs, hs), (rhc, hc)):
                S.activation(rp.full(), src.full(), AF.Copy, scale=2.0, bias=1e-20)
                V.reciprocal_approx_fast(out=rp.full(), in_=rp.full())
            # union0 = w1*h1 + w2*h2  (plain width FB)
            m1 = io.tile([P, FB], f32, tag="m1")
            m2 = io.tile([P, FB], f32, tag="m2")
            union0 = sm.tile([P, FB], f32, tag="union0")
            gtt(m1[:], feat(0, 2), feat(0, 3), OP.mult)
            gtt(m2[:], feat(1, 2), feat(1, 3), OP.mult)
            gtt(union0[:], m1[:], m2[:], OP.add)

            # input tile + prepass scratch no longer needed: free the io pool
            # so the heavy per-column-tile pool can use its SBUF space
            stack.close()
            hv = stack.enter_context(tc.tile_pool(name="heavy", bufs=1))

            lsums = []
            SW = 2 * FT  # stacked width per edge slice

            for t in range(NT):
                c0 = t * FT

                def E(tile4):     # (P, 4, 2, FT) edge/half view of 4*SW tile
                    return tile4[:].rearrange("p (e h f) -> p e h f", e=4, h=2)

                AXt = hv.tile([P, 4 * SW], f32, tag="AXt")
                AYt = hv.tile([P, 4 * SW], f32, tag="AYt")
                INX = hv.tile([P, 4 * SW], f32, tag="INX")
                INY = hv.tile([P, 4 * SW], f32, tag="INY")
                Ut = hv.tile([P, 4 * SW], f32, tag="Ut")
                Vt = hv.tile([P, 4 * SW], f32, tag="Vt")
                NPt = hv.tile([P, 4 * SW], f32, tag="NPt")
                TLX = hv.tile([P, 4 * SW], f32, tag="TLX")

                dXc, dYc = dX.sl(c0, FT), dY.sl(c0, FT)
                g0xc, g0yc = g0x.sl(c0, FT), g0y.sl(c0, FT)
                n1c, n2c = n1.sl(c0, FT), n2.sl(c0, FT)
                Wcc, Hcc = Wc.sl(c0, FT), Hc.sl(c0, FT)

                # corners of the moving box in the fixed box's frame (GpSimd)
                gtt(E(AXt)[:, 0], dXc, g0xc, OP.add)
                gtt(E(AXt)[:, 1], dXc, n1c, OP.subtract)
                gtt(E(AXt)[:, 2], dXc, g0xc, OP.subtract)
                gtt(E(AXt)[:, 3], dXc, n1c, OP.add)
                gtt(E(AYt)[:, 0], dYc, g0yc, OP.add)
                gtt(E(AYt)[:, 1], dYc, n2c, OP.add)
                gtt(E(AYt)[:, 2], dYc, g0yc, OP.subtract)
                gtt(E(AYt)[:, 3], dYc, n2c, OP.subtract)

                # ---- enclosing rect (bbox in each frame, min of the two) ----
                exm = sm.tile([P, SW], f32, tag="exm")
                exn = sm.tile([P, SW], f32, tag="exn")
                exs = sm.tile([P, SW], f32, tag="exs")
                eys = sm.tile([P, SW], f32, tag="eys")
                ex3 = exm[:].rearrange("p (h f) -> p h f", h=2)
                en3 = exn[:].rearrange("p (h f) -> p h f", h=2)
                es3 = exs[:].rearrange("p (h f) -> p h f", h=2)
                ey3 = eys[:].rearrange("p (h f) -> p h f", h=2)
                # (min/max TT are not legal on GpSimd; comparisons stay on V,
                # adds/mults go to GpSimd)
                for crn, ext, clamp, dst3 in ((AXt, exs, Wcc, es3), (AYt, eys, Hcc, ey3)):
                    c4 = E(crn)
                    vtt(ex3, c4[:, 0], c4[:, 1], OP.max)
                    vtt(en3, c4[:, 2], c4[:, 3], OP.max)
                    vtt(ex3, ex3, en3, OP.max)                    # mx
                    vtt(dst3, ex3, clamp, OP.max)                 # max(mx, W)
                    vtt(ex3, c4[:, 0], c4[:, 1], OP.min)
                    vtt(en3, c4[:, 2], c4[:, 3], OP.min)
                    vtt(ex3, ex3, en3, OP.min)                    # mn
                    S.activation(ex3, ex3, AF.Copy, scale=-1.0)   # -mn
                    vtt(ex3, ex3, clamp, OP.max)                  # max(-mn, W)
                    gtt(dst3, dst3, ex3, OP.add)                  # extent
                gtt(exs[:], exs[:], eys[:], OP.mult)              # areaC stacked
                area_c = sm.tile([P, FT], f32, tag="area_c")
                vtt(area_c[:], es3[:, 0], es3[:, 1], OP.min)

                # ---- signed reciprocal planes (edges 0,1 only: edges 2,3
                # come from the point symmetry a2 = 2*delta - a0, d2 = -d0,
                # whose slab roots are m + roots(edge0), m = 2*delta*inv) ----
                rwcc, rwsc = rwc.sl(c0, FT), rws.sl(c0, FT)
                rhsc, rhcc = rhs.sl(c0, FT), rhc.sl(c0, FT)
                for dst, srcs in (
                    (INX, ((rwcc, -1.0), (rhsc, 1.0))),
                    (INY, ((rwsc, -1.0), (rhcc, -1.0))),
                ):
                    d4 = E(dst)
                    for e, (src, sc) in enumerate(srcs):
                        S.activation(d4[:, e], src, AF.Copy, scale=sc)
                t2d = sm.tile([P, SW], f32, tag="t2d")
                t2d3 = t2d[:].rearrange("p (h f) -> p h f", h=2)
                HW2 = 2 * SW

                def H01(t4):
                    return t4[:, 0:HW2]

                def H23(t4):
                    return t4[:, HW2:2 * HW2]

                # ---- slab clip, x axis, edges 0,1 ----
                nWcc, nHcc = nWc.sl(c0, FT), nHc.sl(c0, FT)
                for e in range(2):
                    vtt(E(Ut)[:, e], nWcc, E(AXt)[:, e], OP.subtract)  # -W - ax
                    vtt(E(Vt)[:, e], Wcc, E(AXt)[:, e], OP.subtract)   # W - ax
                vtt(H01(Ut), H01(Ut), H01(INX), OP.mult)               # ta01
                vtt(H01(Vt), H01(Vt), H01(INX), OP.mult)               # tb01
                vtt(H01(TLX), H01(Ut), H01(Vt), OP.min)                # tlo01
                vtt(H01(Ut), H01(Ut), H01(Vt), OP.max)                 # thi01
                S.activation(t2d3, dXc, AF.Copy, scale=2.0)            # 2*dx
                vtt(E(Vt)[:, 0], t2d3, E(INX)[:, 0], OP.mult)          # m0
                vtt(E(Vt)[:, 1], t2d3, E(INX)[:, 1], OP.mult)          # m1
                vtt(H23(TLX), H01(Vt), H01(TLX), OP.add)               # tlo23
                vtt(H23(Ut), H01(Vt), H01(Ut), OP.add)                 # thi23
                # ---- slab clip, y axis, edges 0,1 ----
                for e in range(2):
                    vtt(E(Vt)[:, e], nHcc, E(AYt)[:, e], OP.subtract)  # -H - ay
                    vtt(E(NPt)[:, e], Hcc, E(AYt)[:, e], OP.subtract)  # H - ay
                vtt(H01(Vt), H01(Vt), H01(INY), OP.mult)               # ta01_y
                vtt(H01(NPt), H01(NPt), H01(INY), OP.mult)             # tb01_y
                vtt(H01(INX), H01(Vt), H01(NPt), OP.min)               # tlo01_y
                vtt(H01(Vt), H01(Vt), H01(NPt), OP.max)                # thi01_y
                S.activation(t2d3, dYc, AF.Copy, scale=2.0)            # 2*dy
                vtt(E(NPt)[:, 0], t2d3, E(INY)[:, 0], OP.mult)         # m0_y
                vtt(E(NPt)[:, 1], t2d3, E(INY)[:, 1], OP.mult)         # m1_y
                vtt(H23(INX), H01(NPt), H01(INX), OP.add)              # tlo23_y
                vtt(H23(Vt), H01(NPt), H01(Vt), OP.add)                # thi23_y
                # ---- interval intersect, dt ----
                # t0 = max(tlo_x, tlo_y, 0); t1 = min(thi_x, thi_y, 1)
                V.scalar_tensor_tensor(TLX[:], TLX[:], 0.0, INX[:], OP.max, OP.max)
                V.scalar_tensor_tensor(Ut[:], Ut[:], 1.0, Vt[:], OP.min, OP.min)
                vtt(TLX[:], Ut[:], TLX[:], OP.subtract)                # t1-t0
                S.activation(TLX[:], TLX[:], AF.Relu)                  # dt
                # ---- direction planes, cross(a,d), pieces ----
                wcc, wsc = wc.sl(c0, FT), ws.sl(c0, FT)
                hsc, hcc = hs.sl(c0, FT), hc.sl(c0, FT)
                for dst, srcs in (
                    (INX, ((wcc, -2.0), (hsc, 2.0), (wcc, 2.0), (hsc, -2.0))),   # dx
                    (INY, ((wsc, -2.0), (hcc, -2.0), (wsc, 2.0), (hcc, 2.0))),   # dy
                ):
                    d4 = E(dst)
                    for e, (src, sc) in enumerate(srcs):
                        S.activation(d4[:, e], src, AF.Copy, scale=sc)
                vtt(Vt[:], AXt[:], INY[:], OP.mult)                    # ax*dy
                vtt(Ut[:], AYt[:], INX[:], OP.mult)                    # ay*dx
                vtt(Vt[:], Vt[:], Ut[:], OP.subtract)                  # cad
                vtt(Ut[:], TLX[:], Vt[:], OP.mult)                     # pieces

                # ---- piece sum (stacked), SA (frame-B half) ----
                psS = sm.tile([P, SW], f32, tag="psS")
                ps3 = psS[:].rearrange("p (h f) -> p h f", h=2)
                u4 = E(Ut)
                gtt(ps3, u4[:, 0], u4[:, 1], OP.add)
                gtt(es3, u4[:, 2], u4[:, 3], OP.add)                   # reuse exs
                gtt(ps3, ps3, es3, OP.add)
                dt4 = E(TLX)
                sax = sm.tile([P, FT], f32, tag="sax")
                say = sm.tile([P, FT], f32, tag="say")
                sau = sm.tile([P, FT], f32, tag="sau")
                sav = sm.tile([P, FT], f32, tag="sav")
                st1 = sm.tile([P, FT], f32, tag="st1")
                gtt(sau[:], dt4[:, 2, 0], dt4[:, 0, 0], OP.subtract)
                gtt(sav[:], dt4[:, 3, 0], dt4[:, 1, 0], OP.subtract)
                dx4, dy4 = E(INX), E(INY)
                gtt(sax[:], dx4[:, 2, 0], sau[:], OP.mult)
                gtt(st1[:], dx4[:, 3, 0], sav[:], OP.mult)
                gtt(sax[:], sax[:], st1[:], OP.add)
                gtt(say[:], dy4[:, 2, 0], sau[:], OP.mult)
                gtt(st1[:], dy4[:, 3, 0], sav[:], OP.mult)
                gtt(say[:], say[:], st1[:], OP.add)
                # corr = ddy*(c2*sax - s2*say)... rotated by R2:
                # RSx = c2*sax - s2*say ; RSy = s2*sax + c2*say
                c2v = cS.hsl(0, c0, FT)
                s2v = sS.hsl(0, c0, FT)
                rsx = sm.tile([P, FT], f32, tag="rsx")
                rsy = sm.tile([P, FT], f32, tag="rsy")
                gtt(rsx[:], c2v, sax[:], OP.mult)
                gtt(st1[:], s2v, say[:], OP.mult)
                gtt(rsx[:], rsx[:], st1[:], OP.subtract)
                gtt(rsy[:], s2v, sax[:], OP.mult)
                gtt(st1[:], c2v, say[:], OP.mult)
                gtt(rsy[:], rsy[:], st1[:], OP.add)
                inter = sm.tile([P, FT], f32, tag="inter")
                gtt(inter[:], ddyS.hsl(0, c0, FT), rsx[:], OP.mult)
                gtt(st1[:], ddxS.hsl(0, c0, FT), rsy[:], OP.mult)
                gtt(inter[:], inter[:], st1[:], OP.subtract)           # corr
                gtt(inter[:], inter[:], ps3[:, 0], OP.add)
                gtt(inter[:], inter[:], ps3[:, 1], OP.add)
                S.activation(inter[:], inter[:], AF.Relu, scale=0.5)   # inter area

                # ---- final loss ----
                union = sm.tile([P, FT], f32, tag="union")
                gtt(union[:], union0[:, c0:c0 + FT], inter[:], OP.subtract)
                iou = sm.tile([P, FT], f32, tag="iou")
                V.reciprocal_approx_fast(out=st1[:], in_=union[:])
                gtt(iou[:], inter[:], st1[:], OP.mult)
                V.tensor_scalar(iou[:], iou[:], 1e-6, None, OP.max)
                V.reciprocal_approx_fast(out=st1[:], in_=area_c[:])
                gtt(st1[:], union[:], st1[:], OP.mult)
                rr = sm.tile([P, FT], f32, tag="rr")
                S.activation(rr[:], st1[:], AF.Copy, scale=-1.0, bias=1.0)  # 1-u/ac
                gtt(st1[:], iou[:], iou[:], OP.mult)
                gtt(st1[:], st1[:], iou[:], OP.mult)                   # iou^3
                gtt(iou[:], rr[:], rr[:], OP.mult)
                gtt(iou[:], iou[:], rr[:], OP.mult)                    # r^3
                gtt(st1[:], st1[:], iou[:], OP.subtract)               # giou
                S.activation(st1[:], st1[:], AF.Copy, scale=-1.0, bias=1.0)
                ls = sm.tile([P, 1], f32, tag=f"ls{t}")
                V.tensor_reduce(ls[:], st1[:], AX_.X, OP.add)
                if debug:
                    nc.sync.dma_start(out=dbg_d[0, :, c0:c0 + FT], in_=st1[:])
                    nc.sync.dma_start(out=dbg_d[1, :, c0:c0 + FT], in_=inter[:])
                    nc.sync.dma_start(out=dbg_d[2, :, c0:c0 + FT], in_=union[:])
                    nc.sync.dma_start(out=dbg_d[3, :, c0:c0 + FT], in_=area_c[:])
                lsums.append(ls)

            acc = sm.tile([P, 1], f32, tag="acc")
            gtt(acc[:], lsums[0][:], lsums[1][:], OP.add)
            nc.sync.dma_start(out=out_d[:], in_=acc[:])

    nc.finalize()
    return nc


def _get_nc():
    if "nc" not in _CACHE:
        _CACHE["nc"] = _build()
    return _CACHE["nc"]


def kernel(pred, target):
    from concourse.bass_utils import run_bass_kernel_spmd

    pred = np.ascontiguousarray(np.asarray(pred, dtype=np.float32))
    target = np.ascontiguousarray(np.asarray(target, dtype=np.float32))
    nc = _get_nc()
    in_maps = []
    for i in range(N_CORES):
        sl = slice(i * N_CORE, (i + 1) * N_CORE)
        in_maps.append({"pred": pred[sl], "target": target[sl]})
    res = run_bass_kernel_spmd(nc, in_maps, core_ids=list(range(N_CORES)))
    total = np.float64(0.0)
    for i in range(N_CORES):
        total += np.asarray(res.results[i]["out"], dtype=np.float64).sum()
    return np.float32(total / N_TOTAL)
